# revision 3
# baseline (speedup 1.0000x reference)
"""Trainium2 Bass kernel for MeshInterpolate (interpolate_face_attributes).

Problem (hardcoded shapes):
  pix_to_face [4, 512, 512, 1] int64 (-1 = background), values in [-1, 10000)
  bary_coords [4, 512, 512, 1, 3] f32
  face_memory [10000, 3, 128] f32
  output      [4, 128, 512, 512] f32 (NCHW)

Sharding: data-parallel over (N, H/2): 8 cores, core c handles image c//2,
rows 256*(c%2) .. +256  -> 131072 pixels per core. face_memory replicated.

v2 design (vs v1 at ~512us):
  - pixels are SORTED by face id per core (host-side; host inverse-permutes
    the output for free).  With 131072 draws over 10000 faces every face is
    hit (~13x avg), so the sorted face sequence is near-contiguous.
  - pair-entry gather table: fm2[m] = rows(m//2, (m+1)//2) concatenated
    (1536 B bf16 entries).  One 1536 B descriptor serves TWO sorted pixels
    with faces (f,f) (m=2f) or (f,f+1) (m=2f+1).  Halves SWDGE descriptor
    count AND descriptor-generation ucode time (~400us -> ~170us), and the
    sorted m-sequence turns the HBM gather into a near-sequential sweep.
  - rare pair violations (empty face between two pixels of a pair) are
    split into two half-dummy descriptors (bary 0 for the dummy slot).
  - bary shipped 2-replicated only ([P,48,2] per call): a 2-elem inner run
    is enough for the DVE 2x 16-bit mode (measured 3.36us per 2048-px TT,
    same as the old 8-rep layout; pure stride-0 would block 2x).
  - output int8: psum f32 -> int8 bounce on ACT, host multiplies by the
    global scale s_out.  Output HBM bytes 33.5 -> 16.75 MB/core.
  - PE transpose+vertex-sum unchanged: psum[c,p] += prod_v^T @ I_fp8.
Per-core HBM: gather 100.7 MB + out 16.75 + in ~2.7 => ~120 MB.
"""

import os

import numpy as np

# Safety: recover wedged NeuronCores from a previous crashed process.
os.environ.setdefault("NEURON_RT_RESET_CORES", "1")

P = 128
ELEM = 384            # one face row: 3*128 bf16 elems
PAIR = 2 * ELEM       # pair-entry: 768 bf16 elems (1536 B)
GATHER = 1024         # descriptors per dma_gather call (ring carveout)
PXCALL = 2 * GATHER   # pixels per call (2 px per pair-descriptor)
F = 10000
N_CORES = 8
NPIX_CORE = 131072
NCALLS = 65           # fixed call count; capacity 65*2048=133120 px-slots
NPIX_DEV = NCALLS * PXCALL

_CACHE = {}


def _build_nc(ncalls=NCALLS):
    import concourse.bacc as bacc
    import concourse.mybir as mybir
    from concourse import tile
    from concourse.library_config import mlp

    nc = bacc.Bacc("TRN2", target_bir_lowering=False, debug=False,
                   num_swdge_queues=4)
    fm2 = nc.dram_tensor("fm2", [2 * (F + 1), PAIR], mybir.dt.bfloat16,
                         kind="ExternalInput")
    idxw = nc.dram_tensor("idxw", [ncalls, P, GATHER // 16],
                          mybir.dt.int16, kind="ExternalInput")
    bary2 = nc.dram_tensor("bary2", [ncalls, P, 48, 2], mybir.dt.bfloat16,
                           kind="ExternalInput")
    # fp8 identity: 1.0/0.0 are exact in e4m3, halves PE moving-side reads
    ident = nc.dram_tensor("ident", [P, P], mybir.dt.float8e4, kind="ExternalInput")
    out = nc.dram_tensor("out", [P, ncalls * PXCALL], mybir.dt.int8,
                         kind="ExternalOutput")

    with tile.TileContext(nc) as tc:
        nc.gpsimd.load_library(mlp)
        with (
            tc.tile_pool(name="const", bufs=1) as constp,
            tc.tile_pool(name="io", bufs=6) as iop,
            tc.tile_pool(name="attrs", bufs=6) as attrp,
            tc.tile_pool(name="prod", bufs=3) as prodp,
            tc.tile_pool(name="bounce", bufs=4) as bouncep,
            tc.tile_pool(name="ps", bufs=4, space="PSUM") as psump,
        ):
            id_sb = constp.tile([P, P], mybir.dt.float8e4, tag="ident")
            nc.sync.dma_start(id_sb[:], ident[:])
            for ch in range(ncalls):
                idx_sb = iop.tile([P, GATHER // 16], mybir.dt.int16, tag="idx")
                b2 = iop.tile([P, 48, 2], mybir.dt.bfloat16, tag="bary")
                nc.sync.dma_start(idx_sb[:], idxw[ch])
                nc.sync.dma_start(b2[:], bary2[ch])
                attrs = attrp.tile([P, 8, PAIR], mybir.dt.bfloat16, tag="attrs")
                with tc.high_priority(offset=400):
                    nc.gpsimd.dma_gather(
                        attrs[:], fm2[:], idx_sb[:],
                        GATHER, GATHER, PAIR, elem_step=PAIR,
                        queue_num=ch % 4)
                # prod[p, (grp slot v), c] = attrs * bary_eff; 2-elem inner
                # runs on the bary operand keep the DVE in 2x 16-bit mode
                prod = prodp.tile([P, 48, P], mybir.dt.bfloat16, tag="prod")
                a4 = (attrs[:].rearrange("p g (x c) -> p (g x) c", c=P)
                      .rearrange("p q (a b) -> p q a b", b=2))
                b4 = b2[:].unsqueeze(2).broadcast_to((P, 48, P // 2, 2))
                p4 = prod[:].rearrange("p q (a b) -> p q a b", b=2)
                nc.vector.tensor_tensor(p4, a4, b4, mybir.AluOpType.mult)
                # PE transpose + vertex sum: psum[c, p] += prod_v^T @ I
                for h in range(2):
                    ps = psump.tile([P, GATHER], mybir.dt.float32, tag="ps")
                    for bb in range(8):
                        b = h * 8 + bb
                        for v in range(3):
                            nc.tensor.matmul(
                                ps[:, bb * P:(bb + 1) * P],
                                prod[:, b * 3 + v, :],
                                id_sb[:], start=(v == 0), stop=(v == 2))
                    bounce = bouncep.tile([P, GATHER], mybir.dt.int8,
                                          tag="bounce")
                    nc.scalar.copy(bounce[:], ps[:])
                    col = ch * PXCALL + h * GATHER
                    nc.sync.dma_start(out[:, col:col + GATHER], bounce[:])
    nc.compile()
    return nc


def _get_nc():
    if "nc" not in _CACHE:
        _CACHE["nc"] = _build_nc()
    return _CACHE["nc"]


def _pack_core(idx_cl, bary, inv_sout):
    """Sort pixels by face, pack into pair descriptors.

    Returns (idxw [NCALLS,P,64] i16, bary2 [NCALLS,P,48,2] bf16,
             colmap [NPIX_CORE] i32: device out column of each orig pixel).
    """
    import ml_dtypes
    bf16 = ml_dtypes.bfloat16

    order = np.argsort(idx_cl, kind="stable").astype(np.int32)
    c = idx_cl[order].astype(np.int32)
    a, b = c[0::2], c[1::2]
    d = b - a
    sp = order.reshape(-1, 2)
    viol = np.nonzero(d >= 2)[0]
    if len(viol):
        keep = d < 2
        nv = len(viol)
        m = np.concatenate([(2 * a + d)[keep], 2 * a[viol], 2 * b[viol]])
        sp = np.concatenate([
            sp[keep],
            np.stack([sp[viol, 0], np.full(nv, -1, np.int32)], 1),
            np.stack([sp[viol, 1], np.full(nv, -1, np.int32)], 1)])
    else:
        m = 2 * a + d
    W = len(m)
    cap = NCALLS * GATHER
    assert W <= cap, (W, cap)
    m_pad = np.zeros(cap, np.int32)
    m_pad[:W] = m
    sp_pad = np.full((cap, 2), -1, np.int32)
    sp_pad[:W] = sp

    idxw = np.ascontiguousarray(
        m_pad.astype(np.int16).reshape(NCALLS, GATHER // 16, 16)
        .transpose(0, 2, 1))                      # [NCALLS, 16, 64]
    idxw = np.ascontiguousarray(np.tile(idxw, (1, 8, 1)))

    # bary_eff per (call, part, grp*6+slot*3+v): zero for dummy slots
    be = bary[np.clip(sp_pad, 0, None)] * (sp_pad >= 0)[..., None] * inv_sout
    b_t = (be.reshape(NCALLS, 8, P, 2, 3)
           .transpose(0, 2, 1, 3, 4)
           .reshape(NCALLS, P, 48))
    bary2 = np.ascontiguousarray(
        np.repeat(b_t[..., None], 2, axis=3)).astype(bf16)

    D = np.arange(cap, dtype=np.int64)
    colbase = (D // GATHER) * PXCALL + ((D % GATHER) // P) * 2 * P + (D % P)
    colmap = np.empty(NPIX_CORE, np.int32)
    for s in (0, 1):
        px = sp_pad[:, s]
        valid = px >= 0
        colmap[px[valid]] = (colbase + s * P)[valid].astype(np.int32)
    return idxw, bary2, colmap


def _prep_in_maps(pix_to_face, bary_coords, face_memory):
    import ml_dtypes
    bf16 = ml_dtypes.bfloat16

    N, H, W, K = pix_to_face.shape          # 4, 512, 512, 1
    assert (N, H, W, K) == (4, 512, 512, 1)
    fm = np.asarray(face_memory, dtype=np.float32).reshape(F, ELEM)
    s_out = float(np.abs(fm).max()) * 1.01 / 127.0
    _CACHE["s_out"] = s_out
    rows = np.zeros((F + 2, ELEM), np.float32)
    rows[:F] = fm
    rows = rows.astype(bf16)
    r = np.arange(2 * (F + 1))
    fm2 = np.ascontiguousarray(
        np.concatenate([rows[r // 2], rows[(r + 1) // 2]], axis=1))
    ident = np.eye(P, dtype=np.float32).astype(ml_dtypes.float8_e4m3)

    idx_all = np.asarray(pix_to_face).reshape(N, H, W)
    bary_all = np.asarray(bary_coords, dtype=np.float32).reshape(N, H, W, 3)
    inv_sout = 1.0 / s_out

    in_maps = []
    colmaps = []
    for core in range(N_CORES):
        n, hh = core // 2, (core % 2) * 256
        idx = idx_all[n, hh:hh + 256].reshape(-1)
        bary = bary_all[n, hh:hh + 256].reshape(-1, 3)
        idx_cl = np.where(idx < 0, F, idx).astype(np.int32)
        idxw, bary2, colmap = _pack_core(idx_cl, bary, inv_sout)
        colmaps.append(colmap)
        in_maps.append({"fm2": fm2, "idxw": idxw, "bary2": bary2,
                        "ident": ident})
    _CACHE["colmaps"] = colmaps
    return in_maps


def _assemble(results):
    s_out = _CACHE["s_out"]
    colmaps = _CACHE["colmaps"]
    out_full = np.empty((4, 128, 512, 512), dtype=np.float32)
    for core in range(N_CORES):
        n, hh = core // 2, (core % 2) * 256
        dev = results[core]["out"].astype(np.float32) * s_out
        out_full[n, :, hh:hh + 256, :] = (
            dev[:, colmaps[core]].reshape(128, 256, 512))
    return out_full


def run(in_maps, trace=False, trace_kwargs=None):
    from concourse.bass_utils import run_bass_kernel_spmd

    nc = _get_nc()
    kw = {}
    if trace:
        kw = dict(trace=True, trace_kwargs=trace_kwargs or {})
    return run_bass_kernel_spmd(nc, in_maps, list(range(N_CORES)), **kw)


def kernel(pix_to_face, bary_coords, face_memory):
    in_maps = _prep_in_maps(pix_to_face, bary_coords, face_memory)
    res = run(in_maps)
    return _assemble(res.results)


# revision 5
# speedup vs baseline: 1.5495x; 1.5495x over previous
"""Trainium2 Bass kernel for MeshInterpolate (interpolate_face_attributes).

Problem (hardcoded shapes):
  pix_to_face [4, 512, 512, 1] int64 (-1 = background), values in [-1, 10000)
  bary_coords [4, 512, 512, 1, 3] f32
  face_memory [10000, 3, 128] f32
  output      [4, 128, 512, 512] f32 (NCHW)

Sharding: data-parallel over (N, H/2): 8 cores, core c handles image c//2,
rows 256*(c%2) .. +256  -> 131072 pixels per core. face_memory replicated.

v3 design (v1 ~512us, v2 ~522us):
  - pixels SORTED by face id per core (host side; host inverse-permutes the
    output for free).  131072 draws over 10000 faces => every face is hit
    ~13x, so sorted pixels form long same-face runs.
  - ONE descriptor serves FOUR same-face pixels: gather fetches the 768 B
    face row once; the DVE product reads the same attrs tile four times
    (one tensor_tensor per pixel slot with that slot's bary operand).
    Gather HBM traffic drops 100.7 MB -> ~28 MB/core and descriptor count
    131072 -> ~37K (descriptor-gen ucode union time ~400us -> ~100us).
    Runs shorter than 4 leave dummy slots (bary 0): ~11% slot overhead.
  - bary shipped 2-replicated ([.., 2] inner runs): enough for the DVE 2x
    16-bit mode (measured); pure stride-0 would fall to 1x.
  - output int8: psum f32 -> int8 on ACT; host multiplies by global s_out.
  - PE transpose+vertex-sum unchanged: psum[c,p] += prod_v^T @ I_fp8.
  - input loads batched 4 calls per DMA; output stored once per call
    (HWDGE fixed cost ~0.65us/transfer dominates small transfers).
Per-core HBM: gather ~28 MB + out ~19 MB + in ~3 MB => ~50 MB.
"""

import os

import numpy as np

# Safety: recover wedged NeuronCores from a previous crashed process.
os.environ.setdefault("NEURON_RT_RESET_CORES", "1")

P = 128
ELEM = 384            # one face row: 3*128 bf16 elems (768 B)
K = 4                 # pixels (slots) per descriptor
GATHER = 1024         # descriptors per dma_gather call (ring carveout)
PXCALL = K * GATHER   # pixel slots per call
F = 10000
N_CORES = 8
NPIX_CORE = 131072
LOADB = 4             # calls per input-load batch

_CACHE = {}


def _build_nc(ncalls):
    import concourse.bacc as bacc
    import concourse.mybir as mybir
    from concourse import tile
    from concourse.library_config import mlp

    nc = bacc.Bacc("TRN2", target_bir_lowering=False, debug=False,
                   num_swdge_queues=4)
    fmt = nc.dram_tensor("fmt", [F + 1, ELEM], mybir.dt.bfloat16,
                         kind="ExternalInput")
    idxw = nc.dram_tensor("idxw", [ncalls // LOADB, P, LOADB, GATHER // 16],
                          mybir.dt.int16, kind="ExternalInput")
    # bary per (call, partition, slot, (grp,v), 2-rep)
    baryt = nc.dram_tensor("baryt", [ncalls // LOADB, P, LOADB, K, 24, 2],
                           mybir.dt.bfloat16, kind="ExternalInput")
    # fp8 identity: 1.0/0.0 are exact in e4m3, halves PE moving-side reads
    ident = nc.dram_tensor("ident", [P, P], mybir.dt.float8e4, kind="ExternalInput")
    out = nc.dram_tensor("out", [P, ncalls * PXCALL], mybir.dt.int8,
                         kind="ExternalOutput")

    with tile.TileContext(nc) as tc:
        nc.gpsimd.load_library(mlp)
        with (
            tc.tile_pool(name="const", bufs=1) as constp,
            tc.tile_pool(name="io", bufs=3) as iop,
            tc.tile_pool(name="attrs", bufs=8) as attrp,
            tc.tile_pool(name="prod", bufs=3) as prodp,
            tc.tile_pool(name="bounce", bufs=3) as bouncep,
            tc.tile_pool(name="ps", bufs=4, space="PSUM") as psump,
        ):
            id_sb = constp.tile([P, P], mybir.dt.float8e4, tag="ident")
            nc.sync.dma_start(id_sb[:], ident[:])
            for ch in range(ncalls):
                sup, lane = ch // LOADB, ch % LOADB
                if lane == 0:
                    idx_sb = iop.tile([P, LOADB, GATHER // 16],
                                      mybir.dt.int16, tag="idx")
                    b_sb = iop.tile([P, LOADB, K, 24, 2],
                                    mybir.dt.bfloat16, tag="bary")
                    nc.sync.dma_start(idx_sb[:], idxw[sup])
                    nc.sync.dma_start(b_sb[:], baryt[sup])
                attrs = attrp.tile([P, 8, ELEM], mybir.dt.bfloat16, tag="attrs")
                with tc.high_priority(offset=400):
                    nc.gpsimd.dma_gather(
                        attrs[:], fmt[:], idx_sb[:, lane, :],
                        GATHER, GATHER, ELEM, elem_step=ELEM,
                        queue_num=ch % 4)
                # prod[p, slot, (grp v), c] = attrs * bary_eff[slot]; the
                # 2-elem inner runs keep the DVE in 2x 16-bit mode
                prod = prodp.tile([P, K, 24, P], mybir.dt.bfloat16, tag="prod")
                a4 = (attrs[:].rearrange("p g (x c) -> p (g x) c", c=P)
                      .rearrange("p q (a b) -> p q a b", b=2))
                for s in range(K):
                    b4 = (b_sb[:, lane, s].unsqueeze(2)
                          .broadcast_to((P, 24, P // 2, 2)))
                    p4 = prod[:, s].rearrange("p q (a b) -> p q a b", b=2)
                    nc.vector.tensor_tensor(p4, a4, b4, mybir.AluOpType.mult)
                # PE transpose + vertex sum: psum[c, p] += prod_v^T @ I
                bounce = bouncep.tile([P, PXCALL], mybir.dt.int8, tag="bounce")
                for h in range(4):
                    ps = psump.tile([P, GATHER], mybir.dt.float32, tag="ps")
                    for bb in range(8):
                        blk = h * 8 + bb
                        grp, slot = blk // K, blk % K
                        for v in range(3):
                            nc.tensor.matmul(
                                ps[:, bb * P:(bb + 1) * P],
                                prod[:, slot, grp * 3 + v, :],
                                id_sb[:], start=(v == 0), stop=(v == 2))
                    nc.scalar.copy(bounce[:, h * GATHER:(h + 1) * GATHER],
                                   ps[:])
                col = ch * PXCALL
                nc.sync.dma_start(out[:, col:col + PXCALL], bounce[:])
    nc.compile()
    return nc


def _get_nc():
    if "nc" not in _CACHE:
        _CACHE["nc"] = _build_nc(_CACHE.get("ncalls", 36))
    return _CACHE["nc"]


def _pack_core(idx_cl, bary, inv_sout, ncalls):
    """Sort pixels by face; pack K same-face pixels per descriptor.

    Returns (idx16 [ncalls*GATHER] i16 face ids, boff [ncalls*GATHER, K, 3]
    f32 bary_eff (0 = dummy slot), colmap [NPIX_CORE] i32).
    """
    order = np.argsort(idx_cl, kind="stable").astype(np.int32)
    c = idx_cl[order].astype(np.int64)
    L = np.bincount(c, minlength=F + 1)
    dpf = (L + K - 1) // K                    # descs per face
    nd = int(dpf.sum())
    cap = ncalls * GATHER
    assert nd <= cap, (nd, cap)
    starts = np.concatenate([[0], np.cumsum(L)[:-1]])
    r = np.arange(NPIX_CORE) - starts[c]      # rank within face
    dbefore = np.concatenate([[0], np.cumsum(dpf)[:-1]])
    D = (dbefore[c] + r // K).astype(np.int64)  # desc of each sorted pixel
    slot = (r % K).astype(np.int64)

    idx16 = np.zeros(cap, np.int16)           # pad descs fetch face 0
    idx16[:nd] = np.repeat(np.arange(F + 1), dpf).astype(np.int16)

    be = np.zeros((cap, K, 3), np.float32)
    be[D, slot] = bary[order] * inv_sout      # dummy slots stay 0

    grp = (D % GATHER) // P
    part = D % P
    col = (D // GATHER) * PXCALL + (grp * K + slot) * P + part
    colmap = np.empty(NPIX_CORE, np.int32)
    colmap[order] = col.astype(np.int32)
    return idx16, be, colmap


def _prep_in_maps(pix_to_face, bary_coords, face_memory):
    import ml_dtypes
    bf16 = ml_dtypes.bfloat16

    N, H, W, Kd = pix_to_face.shape          # 4, 512, 512, 1
    assert (N, H, W, Kd) == (4, 512, 512, 1)
    fm = np.asarray(face_memory, dtype=np.float32).reshape(F, ELEM)
    s_out = float(np.abs(fm).max()) * 1.01 / 127.0
    _CACHE["s_out"] = s_out
    rows = np.zeros((F + 1, ELEM), np.float32)
    rows[:F] = fm
    fmt = np.ascontiguousarray(rows.astype(bf16))
    ident = np.eye(P, dtype=np.float32).astype(ml_dtypes.float8_e4m3)

    idx_all = np.asarray(pix_to_face).reshape(N, H, W)
    bary_all = np.asarray(bary_coords, dtype=np.float32).reshape(N, H, W, 3)
    inv_sout = 1.0 / s_out

    # first pass: desc counts -> fixed ncalls (rounded to LOADB multiple)
    cores = []
    ndmax = 0
    for core in range(N_CORES):
        n, hh = core // 2, (core % 2) * 256
        idx = idx_all[n, hh:hh + 256].reshape(-1)
        bary = bary_all[n, hh:hh + 256].reshape(-1, 3)
        idx_cl = np.where(idx < 0, F, idx).astype(np.int32)
        nd = int(((np.bincount(idx_cl, minlength=F + 1) + K - 1) // K).sum())
        ndmax = max(ndmax, nd)
        cores.append((idx_cl, bary))
    ncalls = -(-ndmax // GATHER)
    ncalls = -(-ncalls // LOADB) * LOADB
    _CACHE["ncalls"] = ncalls

    in_maps = []
    colmaps = []
    for idx_cl, bary in cores:
        idx16, be, colmap = _pack_core(idx_cl, bary, inv_sout, ncalls)
        colmaps.append(colmap)
        # idx: per call wrap 16-way, replicate to 128 partitions
        iw = (idx16.reshape(ncalls, GATHER // 16, 16).transpose(0, 2, 1))
        iw = np.tile(iw, (1, 8, 1))            # [ncalls, 128, 64]
        iw = np.ascontiguousarray(
            iw.reshape(ncalls // LOADB, LOADB, P, GATHER // 16)
            .transpose(0, 2, 1, 3))
        # bary_eff: [cap, K, 3] -> [ncalls, P, K, (grp v)] -> batched + 2-rep
        bt = (be.reshape(ncalls, 8, P, K, 3)
              .transpose(0, 2, 3, 1, 4)        # [ncalls, P, K, 8, 3]
              .reshape(ncalls, P, K, 24))
        bt = (bt.reshape(ncalls // LOADB, LOADB, P, K, 24)
              .transpose(0, 2, 1, 3, 4))       # [nsup, P, LOADB, K, 24]
        baryt = np.ascontiguousarray(
            np.repeat(bt[..., None], 2, axis=5)).astype(bf16)
        in_maps.append({"fmt": fmt, "idxw": iw, "baryt": baryt,
                        "ident": ident})
    _CACHE["colmaps"] = colmaps
    return in_maps


def _assemble(results):
    s_out = _CACHE["s_out"]
    colmaps = _CACHE["colmaps"]
    out_full = np.empty((4, 128, 512, 512), dtype=np.float32)
    for core in range(N_CORES):
        n, hh = core // 2, (core % 2) * 256
        dev = results[core]["out"].astype(np.float32) * s_out
        out_full[n, :, hh:hh + 256, :] = (
            dev[:, colmaps[core]].reshape(128, 256, 512))
    return out_full


def run(in_maps, trace=False, trace_kwargs=None):
    from concourse.bass_utils import run_bass_kernel_spmd

    nc = _get_nc()
    kw = {}
    if trace:
        kw = dict(trace=True, trace_kwargs=trace_kwargs or {})
    return run_bass_kernel_spmd(nc, in_maps, list(range(N_CORES)), **kw)


def kernel(pix_to_face, bary_coords, face_memory):
    in_maps = _prep_in_maps(pix_to_face, bary_coords, face_memory)
    res = run(in_maps)
    return _assemble(res.results)
